# revision 13
# baseline (speedup 1.0000x reference)
"""AELoss distributed Bass kernel for TRN2 — v8.

Problem: ebd_batch [16, 544, 128, 128] f32, kpts [16, 20, 17, 2] f32.
  vecs[b,p,k,:] = ebd[b, k*32:(k+1)*32, y(b,p,k), x(b,p,k)]
  means = vecs.mean(parts); pull/push L1 stats -> scalar loss.

v4 vs v3 (baseline 87.8us):
- consts packed into 5 DMAs (was 13); the 700KB A idx-base table replaced
  by an 8-col A8 pattern read via stride-0 broadcast APs (values depend
  only on p%16 and j%8).
- idx critical path shortened: y-floor fused to 5 ops ((shr,max) +
  (mult,add) absorb the min), BASEf/XMf via scalar_tensor_tensor (1 op
  each), idx adds dispatched before the x-floor chain runs.
- 4 priming dummy gathers (one per SWDGE queue) issued at engine start to
  absorb the ~6us IRAM library load + queue init before IDX is ready.
- XbS/XbS5 PSUM->SBUF copies moved to the idle Activation engine.

Measured HW law: SWDGE dma_gather generation ~8.2ns/desc per queue,
round-robin over 4 queues -> ~2.2ns/desc aggregate; 21760 descs/core
(one per (point, channel), 256B min payload) -> ~46us stream wall.

Per-core layout: B_L=2 local batches, P=20 people, D=32.
Main parts 0..15: E[c=kl*32+d, (b*4+g)*20+p]; idx = c_local*256 + 2y + xb,
window = 128 planes.  Part16: E5[c=r*32+d, b*5+j] = vec(b, slot 5r+j, 16, d),
window = 32 planes.
"""

import sys

sys.path.insert(0, "/opt/trn_rl_repo")

import numpy as np
import ml_dtypes

import concourse.mybir as mybir
from concourse.ap import AP
from concourse.bacc import Bacc

F32 = mybir.dt.float32
BF16 = mybir.dt.bfloat16
I32 = mybir.dt.int32
I16 = mybir.dt.int16

B, CH, H, W = 16, 544, 128, 128
D = 32
N_PARTS = 17
P = 20
N_CORES = 8
B_L = B // N_CORES
NG = 4
PLANE = H * W
ELEM = 64

C_PULL = 1.0 / (544.0 * 2.0)
C_PUSH = 1.0 / (12800.0 * 2.0)

NMAIN = 1280
NP16 = 80
NCOL = NMAIN + NP16

# dispatch/extraction order: part16 calls first, then 16 main half-calls.
CALLS = [("p", 0, 0, 0, 0), ("p", 1, 0, 0, 1)]
for _b in range(B_L):
    for _g in range(NG):
        for _h in range(2):
            CALLS.append(("m", _b, _g, _h, (len(CALLS)) % 4))


def _host_consts():
    p = np.arange(128)[:, None]
    m8 = np.arange(8)[None, :]
    A8 = ((m8 * 16 + p % 16) * 256).astype(np.float32)        # [128, 8]
    CF32 = np.concatenate([A8, np.ones((128, 1), np.float32)], axis=1)

    IOTA = np.tile(np.arange(ELEM)[None, :], (128, 10))        # [128, 640]
    c = np.arange(128)
    d = np.arange(D)
    SELF = (c[:, None] % 32 == d[None, :]) / N_PARTS          # [128, 32]
    R5M = np.eye(128) / N_PARTS                                # [128, 128]
    CBF = np.concatenate([IOTA, SELF, R5M], axis=1).astype(ml_dtypes.bfloat16)

    kl4 = np.arange(4)[:, None]
    E4T = (c[None, :] // 32 == kl4).astype(np.float64)         # [4, 128]
    S4T = np.zeros((4, 512))
    for kl in range(4):
        S4T[kl, kl * 128:(kl + 1) * 128] = 1.0
    X5T = np.zeros((4, 512))
    for r in range(4):
        X5T[0, r * 128 + r * 32:r * 128 + (r + 1) * 32] = 1.0
    C4 = np.concatenate([E4T, S4T, X5T], axis=1).astype(ml_dtypes.bfloat16)

    R5B = np.zeros((D, 512), dtype=np.float32)
    for r in range(4):
        R5B[:, r * 128 + r * 32:r * 128 + (r + 1) * 32] = np.eye(D)
    RT = (c[None, :] % 32 == d[:, None]).astype(np.float32)    # [32, 128]
    C32 = np.concatenate([R5B, RT], axis=1).astype(np.float32)

    W8 = np.array([[C_PULL, C_PULL, C_PULL, C_PULL, C_PUSH, C_PUSH, 0.0, 0.0]],
                  dtype=np.float32)
    return dict(CF32=CF32, CBF=CBF, C4=C4, C32=C32, W8=W8)


def _kpts_prep(kpts_shard):
    """[B_L, P, 17, 2] -> [4, 400] f32: V[kl, c*200 + (b*5+g)*20 + p] =
    kpts[b, p, min(4g+kl, 16), c]  (g=4 columns all duplicate part 16)."""
    k_ids = np.minimum(np.arange(P), N_PARTS - 1)
    kp = kpts_shard[:, :, k_ids, :]
    kp = kp.reshape(B_L, P, 5, 4, 2).transpose(3, 4, 0, 2, 1)
    return np.ascontiguousarray(kp.reshape(4, 2 * B_L * 5 * P)).astype(
        np.float32
    )


def build_graph():
    nc = Bacc(num_swdge_queues=4, dynamic_dma_scratch_size=98304)

    ebd = nc.declare_dram_parameter("ebd", [B_L, CH, H, W], F32, isOutput=False)
    kp = nc.declare_dram_parameter("kp", [4, 400], F32, isOutput=False)
    CF32_d = nc.declare_dram_parameter("CF32", [128, 9], F32, isOutput=False)
    CBF_d = nc.declare_dram_parameter("CBF", [128, 800], BF16, isOutput=False)
    C4_d = nc.declare_dram_parameter("C4", [4, 1152], BF16, isOutput=False)
    C32_d = nc.declare_dram_parameter("C32", [32, 640], F32, isOutput=False)
    W8_d = nc.declare_dram_parameter("W8", [1, 8], F32, isOutput=False)
    out_ext = nc.declare_dram_parameter("out", [1], F32, isOutput=True)

    from contextlib import ExitStack

    ctx = ExitStack()
    with ctx:
        ctx.enter_context(nc.allow_low_precision(
            "bf16 extraction: values are single-selected elements (one-hot "
            "sum has one nonzero term), no accumulation error"))
        sb = lambda name, shape, dt=F32: ctx.enter_context(
            nc.sbuf_tensor(name, shape, dt)
        )
        ps = lambda name, shape: ctx.enter_context(
            nc.psum_tensor(name, shape, F32)
        )

        Vt = sb("Vt", [4, 400])
        U32 = sb("U32", [4, 400], I32)
        SH = sb("SH", [4, 400], I32)
        YI = sb("YI", [4, 400], I32)
        Yf = sb("Yf", [4, 400])
        Gg = sb("Gg", [4, 200])
        BASEf = sb("BASEf", [4, 200], BF16)
        XMf = sb("XMf", [4, 200], BF16)
        CF32t = sb("CF32t", [128, 9])
        CBFt = sb("CBFt", [128, 800], BF16)
        C4t = sb("C4t", [4, 1152], BF16)
        C32t = sb("C32t", [32, 640])
        W8t = sb("W8t", [1, 8])
        IDX = sb("IDX", [128, NCOL], I16)
        G = sb("G", [128, 16 * 640 + 2 * 320])
        IDXD = sb("IDXD", [128, 8], I16)
        GD = sb("GD", [128, 4 * ELEM])
        XBb = sb("XBb", [128, 2 * 640], BF16)
        GBb = sb("GBb", [128, 2 * 640], BF16)
        XB5b = sb("XB5b", [128, 2 * 320], BF16)
        GB5b = sb("GB5b", [128, 2 * 320], BF16)
        M1 = sb("M1", [128, 640], BF16)
        P1 = sb("P1", [128, 640], BF16)
        E = sb("E", [128, 160], BF16)
        E5 = sb("E5", [128, 10], BF16)
        Mrep = sb("Mrep", [D, 160])
        DFm = sb("DFm", [128, 80])
        DF5 = sb("DF5", [128, 5])
        T = sb("T", [128, 8])
        FW = sb("FW", [1, 8])
        OUTs = sb("OUTs", [1, 1])

        Bps = ps("Bps", [128, 4 * 512])   # 4 banks, main cols [0:160] each
        Xb = ps("Xb", [128, 512])         # xm broadcast [0:160]
        Mps = ps("Mps", [D, 2 * P])
        PT1 = ps("PT1", [128, 512])
        PT2 = ps("PT2", [128, 512])
        # late-stage matmul outputs: all at PSUM bank starts (matmul PSUM
        # targets must be bank-aligned), temporally disjoint from producers
        B16v = PT1[:, 0:80]               # part16 base broadcast
        X5ps = PT2[:, 0:10]               # part16 xm broadcast
        MBps = Bps[:, 0:160]              # mean broadcast main (after idx adds)
        M5ps = Bps[:, 512:517]            # mean broadcast p16 (after idx adds)
        Fps = Bps[:, 1024:1032]           # final partition sum

        sk = ctx.enter_context(nc.semaphore("sk"))
        s1 = ctx.enter_context(nc.semaphore("s1"))
        s2 = ctx.enter_context(nc.semaphore("s2"))
        sc = ctx.enter_context(nc.semaphore("sc"))
        gds = [ctx.enter_context(nc.semaphore(f"gd{i}"))
               for i in range(len(CALLS))]
        gw = ctx.enter_context(nc.semaphore("gw"))
        gwm = ctx.enter_context(nc.semaphore("gwm"))
        sv = ctx.enter_context(nc.semaphore("sv"))
        sp = ctx.enter_context(nc.semaphore("sp"))
        block = ctx.enter_context(nc.Block())

        # const blob slice views
        E4v = C4t[:, 0:128]
        S4 = lambda lo, hi: C4t[:, 128 + lo:128 + hi]
        X5 = lambda lo, hi: C4t[:, 640 + lo:640 + hi]
        It640 = CBFt[:, 0:640]
        SELv = CBFt[:, 640:672]
        R5Mv = lambda lo, hi: CBFt[:, 672 + lo:672 + hi]
        R5Bv = lambda lo, hi: C32t[:, lo:hi]
        RTv = C32t[:, 512:640]
        A8v = CF32t[:, 0:8]
        OCv = CF32t[:, 8:9]

        MS = {}
        # PE program is static: Bps16 x2, Bps x4, Xb, X5 x4 (=11), then per b
        # g0, res x4, g1, g2, g3, MB, MB5 x4 (=13), then F.
        PS = {"ma0": 19, "mb50": 23, "mb0": 24,
              "ma1": 32, "mb51": 36, "mb1": 37, "F": 38}

        @block.vector
        def _(vec):
            AL = mybir.AluOpType
            cnt = [0]

            def fin(inst):
                inst.then_inc(sv, 1)
                cnt[0] += 1

            def w():
                vec.wait_ge(sv, cnt[0])

            nc.vector.memset(IDXD[:], 0).then_inc(gwm, 1)
            fin(nc.vector.memset(T[:], 0.0))
            vec.wait_ge(sk, 16)

            def floor_half(lo, hi, dst):
                """dst[:, lo:hi] = float(floor(Vt[:, lo:hi] * 128)) via
                mantissa shift: exact for v in [0, 1)."""
                uh = Vt[:, lo:hi].bitcast(I32)
                fin(nc.vector.tensor_scalar(
                    out=U32[:, lo:hi], in0=uh, scalar1=23, scalar2=None,
                    op0=AL.logical_shift_right,
                ))
                w()
                fin(nc.vector.tensor_scalar(
                    out=SH[:, lo:hi], in0=U32[:, lo:hi], scalar1=-1,
                    scalar2=143, op0=AL.mult, op1=AL.add,
                ))
                w()
                fin(nc.vector.tensor_scalar(
                    out=SH[:, lo:hi], in0=SH[:, lo:hi], scalar1=31,
                    scalar2=None, op0=AL.min,
                ))
                w()
                fin(nc.vector.tensor_scalar(
                    out=U32[:, lo:hi], in0=uh, scalar1=0x7FFFFF,
                    scalar2=0x800000, op0=AL.bitwise_and, op1=AL.bitwise_or,
                ))
                w()
                fin(nc.vector.tensor_tensor(
                    out=YI[:, lo:hi], in0=U32[:, lo:hi], in1=SH[:, lo:hi],
                    op=AL.logical_shift_right,
                ))
                w()
                fin(nc.vector.tensor_copy(out=dst, in_=YI[:, lo:hi]))

            floor_half(0, 200, Yf[:, 0:200])       # y
            fin(nc.vector.tensor_scalar(
                out=Gg[:], in0=Vt[:, 200:400], scalar1=0.5, scalar2=None,
                op0=AL.is_ge,
            ))
            w()
            fin(nc.vector.scalar_tensor_tensor(
                out=BASEf[:], in0=Yf[:, 0:200], scalar=2.0, in1=Gg[:],
                op0=AL.mult, op1=AL.add,
            ))
            MS["base"] = cnt[0]

            # part16 idx adds first (need only B16v = sp 2 and A8)
            vec.wait_ge(s2, 16)
            vec.wait_ge(sp, 2)
            a8 = A8v
            for b in range(B_L):
                out16 = IDX[:, NMAIN + b * 40:NMAIN + (b + 1) * 40].rearrange(
                    "p (a c) -> p a c", c=2)
                in016 = AP(a8.tensor, a8.offset, [a8.ap[0], [0, 20], [1, 2]])
                in116 = B16v[:, b * 40:(b + 1) * 40].rearrange(
                    "p (a c) -> p a c", c=2)
                fin(nc.vector.tensor_tensor(
                    out=out16, in0=in016, in1=in116, op=AL.add,
                ))
            MS["idx16"] = cnt[0]
            # main idx adds: per b, two parities
            vec.wait_ge(sp, 6)
            bp0 = Bps[:]
            for b in range(B_L):
                for par in range(2):
                    out = AP(IDX[:].tensor, IDX[:].offset + b * 640 + par,
                             [IDX[:].ap[0], [160, 4], [8, P], [2, 4]])
                    in0 = AP(a8.tensor, a8.offset + par,
                             [a8.ap[0], [0, 4], [0, P], [2, 4]])
                    in1 = AP(bp0.tensor, bp0.offset + b * 80,
                             [bp0.ap[0], [20, 4], [1, P], [512, 4]])
                    fin(nc.vector.tensor_tensor(
                        out=out, in0=in0, in1=in1, op=AL.add
                    ))
                MS[f"idx{b}"] = cnt[0]

            # x floor + xm (off the gather critical path)
            floor_half(200, 400, Yf[:, 200:400])
            w()
            fin(nc.vector.scalar_tensor_tensor(
                out=XMf[:], in0=Gg[:], scalar=-64.0, in1=Yf[:, 200:400],
                op0=AL.mult, op1=AL.add,
            ))
            MS["xm"] = cnt[0]

            MS["eslice"] = {}

            def extract_main(i, b, g, h):
                vec.wait_ge(sc, 2 * (i + 1))
                xcol = (b * 4 + g) * 20 + h * 10
                buf = i % 2
                w()
                fin(nc.vector.tensor_tensor(
                    out=M1[:], in0=It640,
                    in1=XBb[:, buf * 640:buf * 640 + 640], op=AL.is_equal,
                ))
                w()
                fin(nc.vector.tensor_tensor(
                    out=P1[:], in0=GBb[:, buf * 640:buf * 640 + 640],
                    in1=M1[:], op=AL.mult,
                ))
                w()
                fin(nc.vector.tensor_reduce(
                    out=E[:, xcol:xcol + 10],
                    in_=P1[:].rearrange("p (a b) -> p a b", b=ELEM),
                    axis=mybir.AxisListType.X, op=AL.add,
                ))
                MS["eslice"][(b, g, h)] = cnt[0]

            def extract_p16(i, b):
                vec.wait_ge(sc, 2 * (i + 1))
                if i == 0:
                    vec.wait_ge(s2, 32)  # IOTA (CBF blob)
                w()
                fin(nc.vector.tensor_tensor(
                    out=M1[:, 0:320], in0=CBFt[:, 0:320],
                    in1=XB5b[:, b * 320:(b + 1) * 320], op=AL.is_equal,
                ))
                w()
                fin(nc.vector.tensor_tensor(
                    out=P1[:, 0:320], in0=GB5b[:, b * 320:(b + 1) * 320],
                    in1=M1[:, 0:320], op=AL.mult,
                ))
                w()
                fin(nc.vector.tensor_reduce(
                    out=E5[:, b * 5:(b + 1) * 5],
                    in_=P1[:, 0:320].rearrange("p (a b) -> p a b", b=ELEM),
                    axis=mybir.AxisListType.X, op=AL.add,
                ))
                MS[f"e5_{b}"] = cnt[0]

            def tail_b(b):
                vec.wait_ge(sp, PS[f"ma{b}"])
                mp = Mps[:]
                if b == 0:
                    fin(nc.vector.tensor_copy(
                        out=Mrep[:, 0:80].rearrange(
                            "p (a c) -> p a c", a=4),
                        in_=AP(mp.tensor, mp.offset,
                               [mp.ap[0], [0, 4], [1, P]]),
                    ))
                    MS["mrep0"] = cnt[0]
                else:
                    vec.wait_ge(sc, 37)   # Mrep b1 copied by Activation
                in0 = AP(mp.tensor, mp.offset + b * P,
                         [mp.ap[0], [1, P], [0, P]])
                mr = Mrep[:]
                in1 = AP(mr.tensor, mr.offset + b * 80,
                         [mr.ap[0], [0, P], [1, P]])
                pd = G[0:32, b * 400:(b + 1) * 400].rearrange(
                    "p (a c) -> p a c", a=P)
                w()
                fin(nc.vector.tensor_tensor(
                    out=pd, in0=in0, in1=in1, op=AL.subtract
                ))
                w()
                fin(nc.vector.tensor_reduce(
                    out=T[0:32, 4 + b:5 + b], in_=pd,
                    axis=mybir.AxisListType.XY, op=AL.add,
                    apply_absolute_value=True,
                ))
                vec.wait_ge(sp, PS[f"mb5{b}"])
                fin(nc.vector.tensor_tensor(
                    out=DF5[:], in0=E5[:, b * 5:(b + 1) * 5],
                    in1=M5ps[:, 0:5], op=AL.subtract,
                ))
                w()
                fin(nc.vector.tensor_reduce(
                    out=T[:, 2 + b:3 + b], in_=DF5[:],
                    axis=mybir.AxisListType.X, op=AL.add,
                    apply_absolute_value=True,
                ))
                vec.wait_ge(sp, PS[f"mb{b}"])
                fin(nc.vector.tensor_tensor(
                    out=DFm[:], in0=E[:, b * 80:(b + 1) * 80],
                    in1=MBps[:, b * 80:(b + 1) * 80], op=AL.subtract,
                ))
                w()
                fin(nc.vector.tensor_reduce(
                    out=T[:, b:b + 1],
                    in_=DFm[:].rearrange("p (a c) -> p a c", a=4),
                    axis=mybir.AxisListType.XY, op=AL.add,
                    apply_absolute_value=True,
                ))
                MS[f"tail{b}"] = cnt[0]

            for i, (kind, b, g, h, q) in enumerate(CALLS):
                if kind == "m":
                    extract_main(i, b, g, h)
                    if g == 3 and h == 1:
                        tail_b(b)
                else:
                    extract_p16(i, b)
            MS["tdone"] = cnt[0]
            vec.wait_ge(sp, PS["F"])
            vec.wait_ge(s2, 48)
            fin(nc.vector.tensor_tensor(
                out=FW[:], in0=Fps[0:1, :], in1=W8t[:], op=AL.mult
            ))
            w()
            fin(nc.vector.tensor_reduce(
                out=OUTs[:], in_=FW[:], axis=mybir.AxisListType.X, op=AL.add
            ))
            MS["loss"] = cnt[0]

        @block.tensor
        def _(pe):
            pcnt = [0]

            def pfin(inst):
                inst.then_inc(sp, 1)
                pcnt[0] += 1

            bf = BASEf[:]
            xm = XMf[:]
            # part16 base broadcast (needs only BASEf + S4)
            pe.wait_ge(sv, MS["base"])
            pe.wait_ge(s1, 16)
            for b in range(B_L):
                rhs16 = AP(bf.tensor, bf.offset + b * 100 + 80,
                           [bf.ap[0], [1, 5], [5, 4], [0, 2]])
                pfin(nc.tensor.matmul(
                    out=B16v[:, b * 40:(b + 1) * 40],
                    lhsT=S4(0, 128), rhs=rhs16, start=True, stop=True,
                ))
            # main base broadcast
            rhs_main = AP(bf.tensor, bf.offset,
                          [bf.ap[0], [100, 2], [20, 4], [1, 20]])
            for kl in range(4):
                pfin(nc.tensor.matmul(
                    out=Bps[:, kl * 512:kl * 512 + 160],
                    lhsT=S4(kl * 128, (kl + 1) * 128),
                    rhs=rhs_main, start=True, stop=True,
                ))
            # xm broadcasts
            pe.wait_ge(sv, MS["xm"])
            rhs_xm = AP(xm.tensor, xm.offset,
                        [xm.ap[0], [100, 2], [20, 4], [1, 20]])
            pfin(nc.tensor.matmul(
                out=Xb[:, 0:160], lhsT=E4v, rhs=rhs_xm,
                start=True, stop=True,
            ))
            for r in range(4):
                rhs5 = AP(xm.tensor, xm.offset + 80 + 5 * r,
                          [xm.ap[0], [100, 2], [1, 5]])
                pfin(nc.tensor.matmul(
                    out=X5ps, lhsT=X5(r * 128, (r + 1) * 128), rhs=rhs5,
                    start=(r == 0), stop=(r == 3),
                ))
            assert pcnt[0] == 11
            # means + broadcasts per b
            pe.wait_ge(s2, 32)
            pe.wait_ge(s1, 32)
            for b in range(B_L):
                pe.wait_ge(sv, MS["eslice"][(b, 0, 1)])
                pfin(nc.tensor.matmul(
                    out=Mps[:, b * P:(b + 1) * P], lhsT=SELv,
                    rhs=E[:, b * 80:b * 80 + 20],
                    start=True, stop=False,
                ))
                pe.wait_ge(sv, MS[f"e5_{b}"])
                for r in range(4):
                    pfin(nc.tensor.matmul(
                        out=Mps[:, b * P + 5 * r:b * P + 5 * r + 5],
                        lhsT=R5Mv(r * 32, (r + 1) * 32),
                        rhs=E5[:, b * 5:(b + 1) * 5],
                        start=False, stop=False,
                    ))
                for g in range(1, NG):
                    pe.wait_ge(sv, MS["eslice"][(b, g, 1)])
                    pfin(nc.tensor.matmul(
                        out=Mps[:, b * P:(b + 1) * P], lhsT=SELv,
                        rhs=E[:, (b * 4 + g) * 20:(b * 4 + g + 1) * 20],
                        start=False, stop=(g == NG - 1),
                    ))
                assert PS[f"ma{b}"] == pcnt[0]
                if b == 0:
                    pe.wait_ge(sv, MS["mrep0"])
                else:
                    pe.wait_ge(sc, 37)
                for r in range(4):
                    pfin(nc.tensor.matmul(
                        out=M5ps[:, 0:5],
                        lhsT=R5Bv(r * 128, (r + 1) * 128),
                        rhs=Mrep[:, b * 80 + 5 * r:b * 80 + 5 * r + 5],
                        start=(r == 0), stop=(r == 3),
                    ))
                assert PS[f"mb5{b}"] == pcnt[0]
                pfin(nc.tensor.matmul(
                    out=MBps[:, b * 80:(b + 1) * 80], lhsT=RTv,
                    rhs=Mrep[:, b * 80:(b + 1) * 80], start=True, stop=True,
                ))
                assert PS[f"mb{b}"] == pcnt[0]
            pe.wait_ge(sv, MS["tdone"])
            pfin(nc.tensor.matmul(
                out=Fps[0:1, :], lhsT=OCv, rhs=T[:], start=True, stop=True
            ))
            assert PS["F"] == pcnt[0]

        @block.gpsimd
        def _(gpsimd):
            gpsimd.wait_ge(gwm, 1)
            # prime all four SWDGE queues (IRAM load + per-queue init)
            for q in range(4):
                gpsimd.dma_gather(
                    out_ap=GD[:, q * ELEM:(q + 1) * ELEM].rearrange(
                        "p (a b) -> p a b", b=ELEM),
                    in_ap=AP(ebd, 0, [[ELEM, 32768], [1, ELEM]]),
                    idxs_ap=IDXD[:],
                    num_idxs=128, num_idxs_reg=128, elem_size=ELEM,
                    single_packet=False, queue_num=q,
                ).then_inc(gw, 16)
            for i, (kind, b, g, h, q) in enumerate(CALLS):
                if i == 0:
                    gpsimd.wait_ge(sv, MS["idx16"])
                elif kind == "m" and (b, g, h) == (0, 0, 0):
                    gpsimd.wait_ge(sv, MS["idx0"])
                elif kind == "m" and (b, g, h) == (1, 0, 0):
                    gpsimd.wait_ge(sv, MS["idx1"])
                if kind == "m":
                    base = b * CH * PLANE + g * 128 * PLANE
                    in_ap = AP(ebd, base, [[ELEM, 128 * 256], [1, ELEM]])
                    off = (b * 8 + g * 2 + h) * 640
                    out_ap = G[:, off:off + 640].rearrange(
                        "p (a b) -> p a b", b=ELEM)
                    icol = (b * 4 + g) * 160 + h * 80
                    gpsimd.dma_gather(
                        out_ap=out_ap, in_ap=in_ap,
                        idxs_ap=IDX[:, icol:icol + 80],
                        num_idxs=1280, num_idxs_reg=1280, elem_size=ELEM,
                        single_packet=False, queue_num=q,
                    ).then_inc(gds[i], 16)
                else:
                    base = b * CH * PLANE + 512 * PLANE
                    in_ap = AP(ebd, base, [[ELEM, 32 * 256], [1, ELEM]])
                    off = 16 * 640 + b * 320
                    out_ap = G[:, off:off + 320].rearrange(
                        "p (a b) -> p a b", b=ELEM)
                    gpsimd.dma_gather(
                        out_ap=out_ap, in_ap=in_ap,
                        idxs_ap=IDX[:, NMAIN + b * 40:NMAIN + (b + 1) * 40],
                        num_idxs=640, num_idxs_reg=640, elem_size=ELEM,
                        single_packet=False, queue_num=q,
                    ).then_inc(gds[i], 16)

        @block.sync
        def _(sync):
            sync.dma_start(out=Vt[:], in_=kp[:]).then_inc(sk, 16)
            sync.dma_start(out=C4t[:], in_=C4_d[:]).then_inc(s1, 16)
            sync.dma_start(out=C32t[:], in_=C32_d[:]).then_inc(s1, 16)
            sync.wait_ge(sv, MS["loss"])
            sync.dma_start(out=out_ext[:], in_=OUTs[0:1, 0:1]).then_inc(sk, 16)

        @block.scalar
        def _(scalar):
            scalar.dma_start(out=CF32t[:], in_=CF32_d[:]).then_inc(s2, 16)
            scalar.dma_start(out=CBFt[:], in_=CBF_d[:]).then_inc(s2, 16)
            scalar.dma_start(out=W8t[:], in_=W8_d[:]).then_inc(s2, 16)
            # per-call bf16 conversions: XB64 (broadcast target) + G slice.
            # DVE extraction i waits sc >= 2*(i+1).
            scalar.wait_ge(sp, 11)
            for i, (kind, b, g, h, q) in enumerate(CALLS):
                if kind == "p":
                    xs = X5ps
                    in_xb = AP(xs.tensor, xs.offset + b * 5,
                               [xs.ap[0], [1, 5], [0, ELEM]])
                    out_xb = XB5b[:, b * 320:(b + 1) * 320].rearrange(
                        "p (a e) -> p a e", e=ELEM)
                    scalar.copy(out=out_xb, in_=in_xb).then_inc(sc, 1)
                    scalar.wait_ge(gds[i], 16)
                    goff = 16 * 640 + b * 320
                    scalar.copy(
                        out=GB5b[:, b * 320:(b + 1) * 320],
                        in_=G[:, goff:goff + 320],
                    ).then_inc(sc, 1)
                else:
                    if i >= 4:
                        pk, pb, pg, ph, _ = CALLS[i - 2]
                        scalar.wait_ge(sv, MS["eslice"][(pb, pg, ph)])
                    xcol = (b * 4 + g) * 20 + h * 10
                    buf = i % 2
                    xv = Xb[:]
                    in_xb = AP(xv.tensor, xv.offset + xcol,
                               [xv.ap[0], [1, 10], [0, ELEM]])
                    out_xb = XBb[:, buf * 640:buf * 640 + 640].rearrange(
                        "p (a e) -> p a e", e=ELEM)
                    scalar.copy(out=out_xb, in_=in_xb).then_inc(sc, 1)
                    scalar.wait_ge(gds[i], 16)
                    goff = (b * 8 + g * 2 + h) * 640
                    scalar.copy(
                        out=GBb[:, buf * 640:buf * 640 + 640],
                        in_=G[:, goff:goff + 640],
                    ).then_inc(sc, 1)
            # final-batch mean replication off the DVE critical path
            scalar.wait_ge(sp, PS["ma1"])
            mp1 = Mps[:]
            scalar.copy(
                out=Mrep[:, 80:160].rearrange("p (a c) -> p a c", a=4),
                in_=AP(mp1.tensor, mp1.offset + P,
                       [mp1.ap[0], [0, 4], [1, P]]),
            ).then_inc(sc, 1)

    return nc


_CONSTS = None


def _run(ebd_batch: np.ndarray, kpts: np.ndarray, trace: bool = False):
    from concourse.bass_utils import run_bass_kernel_spmd

    global _CONSTS
    if _CONSTS is None:
        _CONSTS = _host_consts()
    consts = _CONSTS

    nc = build_graph()
    nc.finalize()

    in_maps = []
    for c in range(N_CORES):
        sl = slice(c * B_L, (c + 1) * B_L)
        m = dict(
            ebd=np.ascontiguousarray(ebd_batch[sl]).astype(np.float32),
            kp=_kpts_prep(kpts[sl].astype(np.float32)),
            **consts,
        )
        in_maps.append(m)

    res = run_bass_kernel_spmd(
        nc, in_maps, core_ids=list(range(N_CORES)), trace=trace
    )
    total = sum(float(res.results[c]["out"][0]) for c in range(N_CORES))
    return np.float32(total / B), res


def kernel(ebd_batch: np.ndarray, kpts: np.ndarray) -> np.ndarray:
    return _run(ebd_batch, kpts, trace=False)[0]


if __name__ == "__main__":
    np.random.seed(0)
    ebd = np.random.randn(B, CH, H, W).astype(np.float32)
    kk = np.random.rand(B, P, N_PARTS, 2).astype(np.float32)
    print(kernel(ebd, kk))
